# revision 20
# baseline (speedup 1.0000x reference)
"""Trainium2 Bass kernel for fused Llama attention (nn_LlamaAttentionFused).

Reference computation (B=2, S=1024, H=4096, 32 Q heads, 8 KV heads, D=128):
    xq = x @ wq; xk = x @ wk; xv = x @ wv
    rope(xq, xk); causal GQA flash attention; out = attn @ wo

Sharding: 8-way tensor parallel over heads. Core c owns Q heads 4c..4c+3 and
KV head c (GQA groups stay together). Each core computes a full-shape partial
output (its heads' contribution through wo); the host sums the 8 partials.

v3 design (bf16 end-to-end, fp32 PSUM accumulation):
  - Attention work is SPRINKLED into the projection/out-projection matmul
    streams.  The attention phase is dependency/ACT-bound (PE idles ~18us
    per batch when run as its own phase); interleaving its small matmuls
    between projection matmuls hides that latency entirely.
  - Fixed PSUM bank map (manual sub-bank packing via slices):
      bank pq0: q heads 0,1 (2 x 256-token accum)   | proj runs in
      bank pq1: q heads 2,3                         | 256-token chunks
      bank pkv: k | v (2 x 256-token accum)         |
      bank st : scores ping-pong (2 x [128,256] f32)
      bank pa : PV accum ping-pong (2 x [128,129] f32)
      bank tp : transposes ([128, 4, 128] bf16 slots)
      bank po0/po1: out-projection ring
  - Warmup dummy matmuls at the very start get the PE HAM clock-gate to
    8/8 before the first real matmul; first-chunk weight/x DMAs are
    split small and issued from both sync and scalar DGE queues.
  - Scores computed transposed: sT[k,q] = kT_blk-stationary @ qT, so no
    probs transposes are needed for the PV matmul.  Softmax without
    row-max subtraction (|logits| <~ 12, exp fits fp32 comfortably).
  - Denominator for free: V is augmented with a ones column, so the PV
    matmul's last output column accumulates sum_k probs[k,q].
  - RoPE: half-swap via DMA on the gpsimd DGE queue, multiplies on
    gpsimd (keeps ACT/DVE free); host bakes -sin into the sin table.

Device-side layouts (per core):
    xT   [4096, 2048]  x transposed on host (tokens = 2 batches x 1024)
    wq   [4096, 512]   natural (stationary [K=H, M=dims])
    wkv  [4096, 256]   wk|wv column-concat
    wo   [512, 4096]   natural (moving operand)
    cosf [128, 1024]   freqs_cos.T stacked twice on the partition axis
    sinf [128, 1024]   [-freqs_sin.T ; +freqs_sin.T]
    out  [2048, 4096]  partial output (bf16; host sums in fp32)
"""

import numpy as np
import ml_dtypes

import concourse.bass as bass
import concourse.mybir as mybir
import concourse.tile as tile
from concourse import bacc
from concourse.bass_utils import run_bass_kernel_spmd
from concourse.masks import make_identity

F32 = mybir.dt.float32
BF16 = mybir.dt.bfloat16

B = 2
S = 1024          # tokens per batch
H = 4096          # model dim
D = 128           # head dim
HQ = 4            # q heads per core
NT = B * S        # total tokens
SCALE = 1.0 / float(np.sqrt(D))
NEG = -1.0e30     # additive causal mask value (pre-scale)

HC = H // 128     # 32 contraction chunks for the projections
CHUNK = 256       # projection token-chunk
NCH = S // CHUNK  # 4 chunks per batch
WARMUP_MMS = 26   # dummy matmuls to warm the HAM clock gate


def build_program():
    nc = bacc.Bacc("TRN2", target_bir_lowering=False, debug=False, num_devices=8)

    xT = nc.dram_tensor("xT", [H, NT], BF16, kind="ExternalInput").ap()
    wq = nc.dram_tensor("wq", [H, HQ * D], BF16, kind="ExternalInput").ap()
    wkv = nc.dram_tensor("wkv", [H, 2 * D], BF16, kind="ExternalInput").ap()
    wo = nc.dram_tensor("wo", [HQ * D, H], BF16, kind="ExternalInput").ap()
    cosf = nc.dram_tensor("cosf", [128, S], BF16, kind="ExternalInput").ap()
    sinf = nc.dram_tensor("sinf", [128, S], BF16, kind="ExternalInput").ap()
    out = nc.dram_tensor("out", [NT, H], BF16, kind="ExternalOutput").ap()

    xT_r = xT.rearrange("(n p) f -> p n f", p=128)     # [128, 32, 2048]
    wq_r = wq.rearrange("(n p) f -> p n f", p=128)     # [128, 32, 512]
    wkv_r = wkv.rearrange("(n p) f -> p n f", p=128)   # [128, 32, 256]
    wo_r = wo.rearrange("(n p) f -> p n f", p=128)     # [128, 4, 4096]

    with tile.TileContext(nc) as tc:
        with (
            tc.tile_pool(name="const", bufs=1) as const,
            tc.tile_pool(name="weights", bufs=1) as weights,
            tc.tile_pool(name="stream", bufs=5) as stream,
            tc.tile_pool(name="acts", bufs=1) as acts,
            tc.tile_pool(name="vts", bufs=2) as vts,
            tc.tile_pool(name="ropes", bufs=4) as ropes,
            tc.tile_pool(name="probs", bufs=2) as probs,
            tc.tile_pool(name="an", bufs=6) as anpool,
            tc.tile_pool(name="ev", bufs=2) as evpool,
            tc.tile_pool(name="stats", bufs=16) as stats,
            tc.tile_pool(name="ps", bufs=1, space="PSUM") as psum,
        ):
            # ---- tiny constant first: warmup matmul source -----------------
            warm = const.tile([128, 128], BF16)
            nc.gpsimd.memset(warm, 0.25)

            # ---- PSUM bank map (all persistent, manually sliced) -----------
            pq0 = psum.tile([128, 512], F32, tag="pq0")
            pq1 = psum.tile([128, 512], F32, tag="pq1")
            pkv = psum.tile([128, 512], F32, tag="pkv")
            stb = psum.tile([128, 512], F32, tag="st")
            pab = psum.tile([128, 512], F32, tag="pa")
            tpb = psum.tile([128, 4, 128], BF16, tag="tp")
            # po ring: 2 banks via tag bufs
            PA_SL = [slice(0, D + 1), slice(256, 256 + D + 1)]

            # ---- warmup: keep PE busy from the end of the preamble so the
            # HAM clock-gate reaches 8/8 before the first real matmul --------
            for i in range(WARMUP_MMS):
                nc.tensor.matmul(pq0[:, 0:128], warm, warm,
                                 start=True, stop=True)

            # ---- constants -------------------------------------------------
            ident = const.tile([128, 128], BF16)
            make_identity(nc, ident)

            # maskT[p, f] = 0 where p <= f (k <= q valid), NEG where k > q
            maskT = const.tile([128, 128], F32)
            nc.gpsimd.memset(maskT, 0.0)
            nc.gpsimd.affine_select(
                out=maskT,
                in_=maskT,
                compare_op=mybir.AluOpType.is_ge,
                fill=NEG,
                base=0,
                pattern=[[1, 128]],
                channel_multiplier=-1,
            )

            # ---- resident weights (DMAs emitted just-in-time) --------------
            wq_s = weights.tile([128, HC, HQ * D], BF16)
            wkv_s = weights.tile([128, HC, 2 * D], BF16)
            cosf_s = const.tile([128, S], BF16)
            sinf_s = const.tile([128, S], BF16)
            wo_s = weights.tile([128, HQ, H], BF16)

            # ---- persistent activations ------------------------------------
            qTs, kTs, vnats, attnTs = [], [], [], []
            for b in range(B):
                qTs.append(acts.tile([128, HQ, S], BF16, tag=f"qT{b}",
                                     name=f"qT{b}"))
                kTs.append(acts.tile([128, S], BF16, tag=f"kT{b}",
                                     name=f"kT{b}"))
                vnats.append(acts.tile([128, S // 128, D + 1], BF16,
                                       tag=f"vnat{b}", name=f"vnat{b}"))
                attnTs.append(acts.tile([128, HQ, S], BF16, tag=f"attnT{b}",
                                        name=f"attnT{b}"))
            for b in range(B):
                nc.gpsimd.memset(vnats[b], 1.0)  # ones column for the denom

            # ============================================================
            # attention step generators
            # ============================================================
            def rope_swap(dst, nh, tsl):
                """Half-swap of dst[:, 0:nh, tsl] into a fresh scratch
                tile via a small DMA pair (partition halves cross)."""
                scr = ropes.tile([128, HQ, CHUNK], BF16, tag="scr")
                nc.sync.dma_start(out=scr[0:64, 0:nh, :],
                                  in_=dst[64:128, 0:nh, tsl])
                nc.sync.dma_start(out=scr[64:128, 0:nh, :],
                                  in_=dst[0:64, 0:nh, tsl])
                return scr

            def rope_muls(dst, d, tsl, scr):
                nc.gpsimd.tensor_mul(dst[:, d, tsl], dst[:, d, tsl],
                                     cosf_s[:, tsl])
                nc.gpsimd.tensor_mul(scr[:, d, :], scr[:, d, :],
                                     sinf_s[:, tsl])
                nc.gpsimd.tensor_add(dst[:, d, tsl], dst[:, d, tsl],
                                     scr[:, d, :])

            def rope_chunk(dst, nh, tsl):
                """RoPE on dst[:, 0:nh, tsl] in place ([128, nh, 256] bf16).
                Multiplies on gpsimd (SBUF-only engine, keeps DVE free)."""
                scr = rope_swap(dst, nh, tsl)
                for d in range(nh):
                    rope_muls(dst, d, tsl, scr)

            st_ctr = [0]
            pa_ctr = [0]

            def attn_unit_steps(b, hh, qc):
                """Yield once per small PE step for attention unit
                (batch b, head hh, 512-token q-chunk qc).  Steps are sized
                so the host stream provides >= ~0.6us between consecutive
                pulls, giving ACT/DVE time to keep the PE fed."""
                pt = probs.tile([128, S // 128, 512], BF16, tag="pt")
                q0 = qc * 512

                def score1(kc, w):
                    """one score matmul for q-window w (256 cols inside
                    qc) and k-block kc; mask + exp evacuation follow."""
                    qw0 = q0 + w * 256
                    qoff = max(0, kc * 128 - qw0)  # 0 or 128 (diag)
                    sl = st_ctr[0] % 2
                    st_ctr[0] += 1
                    st = stb[:, sl * 256:sl * 256 + 256]
                    nc.tensor.matmul(
                        st[:, qoff:256],
                        kTs[b][:, kc * 128:(kc + 1) * 128],
                        qTs[b][:, hh, qw0 + qoff:qw0 + 256],
                        start=True, stop=True,
                    )
                    if kc * 128 >= qw0:  # diagonal block: causal mask
                        nc.vector.tensor_add(
                            st[:, qoff:qoff + 128],
                            st[:, qoff:qoff + 128],
                            maskT,
                        )
                    nc.scalar.activation(
                        pt[:, kc, w * 256 + qoff:w * 256 + 256],
                        st[:, qoff:256],
                        mybir.ActivationFunctionType.Exp,
                        scale=SCALE,
                    )

                def pv(qr):
                    """PV accumulation for q-block qr (128 q cols inside
                    qc), then renorm by the free denominator column."""
                    qb = qc * 4 + qr
                    sl = pa_ctr[0] % 2
                    pa_ctr[0] += 1
                    pa = pab[:, PA_SL[sl]]
                    for kc in range(qb + 1):
                        nc.tensor.matmul(
                            pa,
                            pt[:, kc, qr * 128:(qr + 1) * 128],
                            vnats[b][:, kc, :],
                            start=(kc == 0), stop=(kc == qb),
                        )
                    rec = stats.tile([128, 1], F32, tag="stt")
                    nc.vector.reciprocal(rec, pa[:, D:D + 1])
                    an = anpool.tile([128, 128], BF16, tag="an")
                    nc.vector.tensor_scalar_mul(an, pa[:, 0:D], rec)
                    return an

                def tp2(ans0, ans1):
                    # attn transposes own tp slots 2,3 (v-transposes use 0,1)
                    nc.tensor.transpose(tpb[:, 2, :], ans0, ident)
                    nc.tensor.transpose(tpb[:, 3, :], ans1, ident)

                def cp2(qr):
                    nc.vector.tensor_copy(
                        attnTs[b][:, hh, q0 + qr * 128:q0 + (qr + 2) * 128],
                        tpb[:, 2:4, :],
                    )

                ans = {}
                if qc == 0:
                    # plan: S(w0:k0,k1)  S(w1:k0..k3)  PV qr0..3
                    plan = [
                        lambda: score1(0, 0), lambda: score1(1, 0),
                        lambda: ans.__setitem__(0, pv(0)),
                        lambda: score1(0, 1),
                        lambda: ans.__setitem__(1, pv(1)),
                        lambda: score1(1, 1),
                        lambda: tp2(ans[0], ans[1]),
                        lambda: score1(2, 1),
                        lambda: cp2(0),
                        lambda: score1(3, 1),
                        lambda: ans.__setitem__(2, pv(2)),
                        lambda: ans.__setitem__(3, pv(3)),
                        lambda: tp2(ans[2], ans[3]),
                        lambda: cp2(2),
                    ]
                else:
                    # windows w0 (kc 0..5), w1 (kc 0..7)
                    plan = (
                        [lambda kc=kc: score1(kc, 0) for kc in range(6)]
                        + [lambda: ans.__setitem__(0, pv(0)),
                           lambda: score1(0, 1),
                           lambda: ans.__setitem__(1, pv(1))]
                        + [lambda kc=kc: score1(kc, 1) for kc in range(1, 7)]
                        + [lambda: ans.__setitem__(2, pv(2)),
                           lambda: tp2(ans[0], ans[1]),
                           lambda: score1(7, 1),
                           lambda: ans.__setitem__(3, pv(3)),
                           lambda: cp2(0),
                           lambda: tp2(ans[2], ans[3]),
                           lambda: cp2(2)]
                    )
                for stepfn in plan:
                    stepfn()
                    yield

            # ---- injector: pulls attention steps into host streams --------
            class Inject:
                def __init__(self):
                    self.q = []
                    self.acc = 0.0
                    self.skip = 0

                def add(self, *its):
                    self.q.extend(its)

                def delay(self, n):
                    self.skip += n

                def pull(self, rate):
                    if self.skip > 0:
                        self.skip -= 1
                        return
                    self.acc += rate
                    while self.acc >= 1.0:
                        self.acc -= 1.0
                        while self.q:
                            try:
                                next(self.q[0])
                                break
                            except StopIteration:
                                self.q.pop(0)
                        else:
                            return

                def flush(self):
                    while self.q:
                        try:
                            next(self.q[0])
                        except StopIteration:
                            self.q.pop(0)

            inj = Inject()

            # ============================================================
            # projection: qT/kT/vT = w.T @ x, in 256-token chunks
            # ============================================================
            evac_prev = [None]   # deferred evacuation closure

            def filler(n):
                # dummy matmuls into a po bank: keeps the PE (and the HAM
                # activity window) busy across gaps that have no attention
                # work to pull; po banks are unused until the out phase.
                fil = psum.tile([128, 512], F32, tag="po", bufs=2,
                                name="fil")
                for _ in range(n):
                    nc.tensor.matmul(fil[:, 0:128], warm, warm,
                                     start=True, stop=True)

            def run_evac():
                if evac_prev[0] is not None:
                    ev_fn = evac_prev[0]
                    evac_prev[0] = None
                    ev_fn()

            def proj_evac(b, c, rate):
                """Evacuate + rope the accumulators of chunk (b, c).
                Emitted at the START of the next chunk (after its xp
                triggers) so the sync queue is never blocked and the
                next chunk's matmuls only wait on the qT copies."""
                tsl = slice(c * CHUNK, (c + 1) * CHUNK)
                kT, qT = kTs[b], qTs[b]
                nc.vector.tensor_copy(kT[:, tsl], pkv[:, 0:256])
                nc.scalar.copy(qT[:, 0, tsl], pq0[:, 0:256])
                nc.scalar.copy(qT[:, 1, tsl], pq0[:, 256:512])
                nc.vector.tensor_copy(qT[:, 2, tsl], pq1[:, 0:256])
                nc.vector.tensor_copy(qT[:, 3, tsl], pq1[:, 256:512])
                rope_chunk(kT.rearrange("p (a f) -> p a f", a=1), 1, tsl)
                rope_chunk(qT, HQ, tsl)
                inj.pull(rate)
                vT = vts.tile([128, CHUNK], BF16, tag="vT")
                nc.vector.tensor_copy(vT, pkv[:, 256:512])
                # v natural [tok, d] via PE transpose (tp slots 0,1)
                for i in range(2):
                    nc.tensor.transpose(tpb[:, i, :],
                                        vT[:, i * 128:(i + 1) * 128], ident)
                nc.vector.tensor_copy(
                    vnats[b][:, c * 2:c * 2 + 2, 0:128], tpb[:, 0:2, :])
                inj.pull(rate)

            def evac_first():
                # evacuate the 512-token first pass (banks: pq0=h0, pq1=h1,
                # st=h2, pa=h3, pkv=k, po0=v).  Rope runs head-major so the
                # first attention unit (head 0) unblocks earliest.
                kT, qT = kTs[0], qTs[0]
                kTv = kT.rearrange("p (a f) -> p a f", a=1)
                h0, h1 = slice(0, 256), slice(256, 512)
                nc.vector.tensor_copy(kT[:, 0:512], pkv[:, 0:512])
                nc.scalar.copy(qT[:, 0, 0:512], pq0)
                nc.scalar.copy(qT[:, 1, 0:512], pq1)
                nc.vector.tensor_copy(qT[:, 2, 0:512], stb)
                nc.vector.tensor_copy(qT[:, 3, 0:512], pab)
                # v first: PV steps need vnat early
                vT2 = vts.tile([128, 2, CHUNK], BF16, tag="vT2")
                nc.vector.tensor_copy(vT2, po_first[0])
                vT2f = vT2.rearrange("p a f -> p (a f)")
                for i in range(4):
                    nc.tensor.transpose(tpb[:, i, :],
                                        vT2f[:, i * 128:(i + 1) * 128],
                                        ident)
                nc.vector.tensor_copy(vnats[0][:, 0:4, 0:128], tpb)
                sck0 = rope_swap(kTv, 1, h0)
                sck1 = rope_swap(kTv, 1, h1)
                scq0 = rope_swap(qT, HQ, h0)
                scq1 = rope_swap(qT, HQ, h1)
                rope_muls(kTv, 0, h0, sck0)
                rope_muls(kTv, 0, h1, sck1)
                for d in range(HQ):
                    rope_muls(qT, d, h0, scq0)
                    rope_muls(qT, d, h1, scq1)

            po_first = []

            def proj_first():
                """First 512 tokens of batch 0 as one N=512 pass: halves
                the JIT weight-bandwidth demand (the startup is DMA-bound)
                and keeps the PE stream dense so the HAM clock-gate warms
                once and stays warm.  Uses 6 banks; attention (which needs
                st/pa/tp) cannot start before this finishes anyway."""
                pov = psum.tile([128, 512], F32, tag="po", bufs=2, name="pov")
                po_first.append(pov)
                xpfs = []
                for g in range(8):              # 8 x tiles of 4 hc each
                    xpf = stream.tile([128, 4, 512], BF16, tag="xpf",
                                      bufs=2, name=f"xpf{g}")
                    xpfs.append(xpf)
                    nc.sync.dma_start(
                        out=xpf, in_=xT_r[:, 4 * g:4 * g + 4, 0:512])
                    eng = nc.sync if g % 2 == 1 else nc.scalar
                    eng.dma_start(out=wq_s[:, 4 * g:4 * g + 4, :],
                                  in_=wq_r[:, 4 * g:4 * g + 4, :])
                    nc.scalar.dma_start(out=wkv_s[:, 4 * g:4 * g + 4, :],
                                        in_=wkv_r[:, 4 * g:4 * g + 4, :])
                    if g == 4:
                        nc.scalar.dma_start(out=cosf_s, in_=cosf)
                        nc.scalar.dma_start(out=sinf_s, in_=sinf)
                for g in range(8):
                    xpf = xpfs[g]
                    for hh in range(4):
                        hc = 4 * g + hh
                        fst, lst = hc == 0, hc == HC - 1
                        for bank, wsl in ((pq0, slice(0, 128)),
                                          (pq1, slice(128, 256)),
                                          (stb, slice(256, 384)),
                                          (pab, slice(384, 512))):
                            nc.tensor.matmul(bank, wq_s[:, hc, wsl],
                                             xpf[:, hh, :],
                                             start=fst, stop=lst)
                        nc.tensor.matmul(pkv, wkv_s[:, hc, 0:128],
                                         xpf[:, hh, :], start=fst, stop=lst)
                        nc.tensor.matmul(pov, wkv_s[:, hc, 128:256],
                                         xpf[:, hh, :], start=fst, stop=lst)
                evac_prev[0] = evac_first

            def proj_chunk(b, c, rate):
                tok0 = b * S + c * CHUNK
                first = (b == 0 and c == 0)
                xps = []
                for g in range(4):              # 4 x-stream tiles of 8 hc
                    xp = stream.tile([128, 8, CHUNK], BF16, tag="xp",
                                     name=f"xp{b}{c}{g}")
                    xps.append(xp)
                    if first:
                        # fine-grained JIT loads alternating DGE queues so
                        # neither queue falls behind the matmul stream
                        nc.sync.dma_start(
                            out=xp[:, 0:2, :],
                            in_=xT_r[:, 8 * g:8 * g + 2, tok0:tok0 + CHUNK])
                        nc.scalar.dma_start(
                            out=xp[:, 2:8, :],
                            in_=xT_r[:, 8 * g + 2:8 * g + 8,
                                     tok0:tok0 + CHUNK])
                        for hh in range(4):
                            hc0 = 8 * g + 2 * hh
                            eng = nc.sync if hh % 2 == 0 else nc.scalar
                            eng.dma_start(out=wq_s[:, hc0:hc0 + 2, :],
                                          in_=wq_r[:, hc0:hc0 + 2, :])
                        eng = nc.sync if g % 2 == 0 else nc.scalar
                        eng.dma_start(
                            out=wkv_s[:, 8 * g:8 * g + 8, :],
                            in_=wkv_r[:, 8 * g:8 * g + 8, :])
                    else:
                        nc.sync.dma_start(
                            out=xp,
                            in_=xT_r[:, 8 * g:8 * g + 8, tok0:tok0 + CHUNK])
                if first:
                    # cos/sin last: not needed until the first evacuation
                    nc.scalar.dma_start(out=cosf_s, in_=cosf)
                    nc.scalar.dma_start(out=sinf_s, in_=sinf)
                had_evac = evac_prev[0] is not None
                run_evac()                      # previous chunk's evacuation
                if had_evac and rate == 0.0:
                    filler(8)   # cover the evac->matmul latency
                for g in range(4):
                    xp = xps[g]
                    for hh in range(8):
                        hc = 8 * g + hh
                        fst, lst = hc == 0, hc == HC - 1
                        # NOTE: start=True clears has_written for the WHOLE
                        # bank, so only the first matmul touching each bank
                        # may set it; the second half relies on per-element
                        # has_written bits (clear after the bank wipe).
                        nc.tensor.matmul(pq0[:, 0:256],
                                         wq_s[:, hc, 0:128], xp[:, hh, :],
                                         start=fst, stop=lst)
                        nc.tensor.matmul(pq0[:, 256:512],
                                         wq_s[:, hc, 128:256], xp[:, hh, :],
                                         start=False, stop=lst)
                        nc.tensor.matmul(pq1[:, 0:256],
                                         wq_s[:, hc, 256:384], xp[:, hh, :],
                                         start=fst, stop=lst)
                        nc.tensor.matmul(pq1[:, 256:512],
                                         wq_s[:, hc, 384:512], xp[:, hh, :],
                                         start=False, stop=lst)
                        nc.tensor.matmul(pkv[:, 0:256],
                                         wkv_s[:, hc, 0:128], xp[:, hh, :],
                                         start=fst, stop=lst)
                        nc.tensor.matmul(pkv[:, 256:512],
                                         wkv_s[:, hc, 128:256], xp[:, hh, :],
                                         start=False, stop=lst)
                        if hh % 2 == 1:
                            inj.pull(rate)
                evac_prev[0] = lambda: proj_evac(b, c, rate)

            # ============================================================
            # output projection: out[tok, :] += attnT.T @ wo
            # ============================================================
            def out_batch(b, rate):
                tok0 = b * S
                attnT = attnTs[b]
                for tb in range(S // 128):
                    ev = evpool.tile([128, H], BF16, tag="ev")
                    for ncol in range(8):
                        po = psum.tile([128, 512], F32, tag="po", bufs=2)
                        for d in range(HQ):
                            nc.tensor.matmul(
                                po,
                                attnT[:, d, tb * 128:(tb + 1) * 128],
                                wo_s[:, d, ncol * 512:(ncol + 1) * 512],
                                start=(d == 0), stop=(d == HQ - 1),
                            )
                        if ncol % 2 == 0:
                            nc.scalar.copy(ev[:, ncol * 512:(ncol + 1) * 512],
                                           po)
                        else:
                            nc.vector.tensor_copy(
                                ev[:, ncol * 512:(ncol + 1) * 512], po)
                        inj.pull(rate)
                    # split across DMA queues; last tile finer for the tail
                    nsp = 4 if (b == B - 1 and tb == S // 128 - 1) else 2
                    w = H // nsp
                    for i in range(nsp):
                        nc.sync.dma_start(
                            out=out[tok0 + tb * 128: tok0 + (tb + 1) * 128,
                                    i * w:(i + 1) * w],
                            in_=ev[:, i * w:(i + 1) * w],
                        )

            # ============================================================
            # schedule
            # ============================================================
            # proj b0 tokens 0-511: single 512-wide pass (DMA-bound start)
            proj_first()
            # proj b0 c2,c3 host attn(b0) qc0: 4 units x 11 steps
            inj.add(*[attn_unit_steps(0, hh, 0) for hh in range(HQ)])
            inj.delay(4)
            proj_chunk(0, 2, 1.8)
            proj_chunk(0, 3, 1.6)
            # proj b1 hosts attn(b0) qc1 (4 x 15) then attn(b1) qc0 (4 x 11)
            inj.add(*[attn_unit_steps(0, hh, 1) for hh in range(HQ)])
            proj_chunk(1, 0, 2.0)
            # wo lands during proj b1, on the scalar DGE queue
            for i in range(0, 4):
                nc.scalar.dma_start(
                    out=wo_s[:, :, i * 512:(i + 1) * 512],
                    in_=wo_r[:, :, i * 512:(i + 1) * 512])
            proj_chunk(1, 1, 2.0)
            inj.add(*[attn_unit_steps(1, hh, 0) for hh in range(HQ)])
            for i in range(4, 8):
                nc.scalar.dma_start(
                    out=wo_s[:, :, i * 512:(i + 1) * 512],
                    in_=wo_r[:, :, i * 512:(i + 1) * 512])
            proj_chunk(1, 2, 2.3)
            proj_chunk(1, 3, 2.3)
            run_evac()          # final chunk's evacuation
            # attn b0 must be fully emitted before out b0 touches attnT[0];
            # remaining b1-qc0 steps drain here too (their deps are long met)
            inj.flush()
            inj.add(*[attn_unit_steps(1, hh, 1) for hh in range(HQ)])
            # out b0 hosts attn(b1) qc1 (4 x 15 steps / 64 pulls)
            out_batch(0, 1.5)
            inj.flush()
            out_batch(1, 0.0)

    nc.compile()
    return nc


_NC = None


def _get_nc():
    global _NC
    if _NC is None:
        _NC = build_program()
    return _NC


def make_in_maps(x, wq, wk, wv, wo, freqs_cos, freqs_sin):
    bf = ml_dtypes.bfloat16
    x = np.asarray(x, np.float32)
    xT = np.ascontiguousarray(x.reshape(NT, H).T.astype(bf))
    cosT = np.asarray(freqs_cos, np.float32).T
    sinT = np.asarray(freqs_sin, np.float32).T
    cosf = np.ascontiguousarray(np.concatenate([cosT, cosT], 0).astype(bf))
    sinf = np.ascontiguousarray(np.concatenate([-sinT, sinT], 0).astype(bf))
    wq = np.asarray(wq, np.float32).astype(bf)
    wk = np.asarray(wk, np.float32).astype(bf)
    wv = np.asarray(wv, np.float32).astype(bf)
    wo = np.asarray(wo, np.float32).astype(bf)
    in_maps = []
    for c in range(8):
        in_maps.append({
            "xT": xT,
            "wq": np.ascontiguousarray(wq[:, c * 512:(c + 1) * 512]),
            "wkv": np.ascontiguousarray(
                np.concatenate([wk[:, c * 128:(c + 1) * 128],
                                wv[:, c * 128:(c + 1) * 128]], axis=1)),
            "wo": np.ascontiguousarray(wo[c * 512:(c + 1) * 512, :]),
            "cosf": cosf,
            "sinf": sinf,
        })
    return in_maps


def kernel(x, wq, wk, wv, wo, freqs_cos, freqs_sin, start_pos=0, **_):
    nc = _get_nc()
    in_maps = make_in_maps(x, wq, wk, wv, wo, freqs_cos, freqs_sin)
    res = run_bass_kernel_spmd(nc, in_maps, list(range(8)))
    acc = res.results[0]["out"].astype(np.float32)
    for c in range(1, 8):
        acc = acc + res.results[c]["out"].astype(np.float32)
    return acc.reshape(B, S, H)


# revision 26
# speedup vs baseline: 1.0060x; 1.0060x over previous
"""Trainium2 Bass kernel for fused Llama attention (nn_LlamaAttentionFused).

Reference computation (B=2, S=1024, H=4096, 32 Q heads, 8 KV heads, D=128):
    xq = x @ wq; xk = x @ wk; xv = x @ wv
    rope(xq, xk); causal GQA flash attention; out = attn @ wo

Sharding: 8-way tensor parallel over heads. Core c owns Q heads 4c..4c+3 and
KV head c (GQA groups stay together). Each core computes a full-shape partial
output (its heads' contribution through wo); the host sums the 8 partials.

v8 design (bf16 end-to-end, fp32 PSUM accumulation):
  - Attention matmuls are SPRINKLED into the projection / out-projection
    streams: run standalone, attention is dependency/ACT-bound and the PE
    idles ~18us per batch; interleaved, that latency hides completely.
  - Projections keep 512-token chunks (256-token variants double the
    JIT-weight bandwidth demand past what the DMA engines deliver at
    startup: each transfer runs on ONE engine at ~21GB/s, concurrency
    comes only from many outstanding transfers).
  - PSUM bank map (8 banks, manually packed):
      psq0..psq3  q-head accumulators (proj) / out-proj ring (psq0-2)
      psk, psv    k/v accumulators
      stb         scores ping-pong (2 x [128,256] f32)
      pab         [0:129] pa_A | [136:264] transpose slot | [272:401] pa_B
  - Scores transposed (sT[k,q] = kT_blk @ qT) so probs feed PV directly;
    denominator comes free via a ones-column in V; no row-max (|logits|
    small).  start=True clears has_written for the WHOLE bank, so only
    the first matmul touching a bank sets it.
  - Warmup matmuls get the HAM clock gate to 8/8 before real work; the
    first x/weight tiles are split small across both DGE queues.
  - RoPE: half-swap via small DMAs, multiplies on gpsimd, head-major
    order so attention unit 0 unblocks earliest.

Device-side layouts (per core):
    xT   [4096, 2048]  x transposed on host (tokens = 2 batches x 1024)
    wq   [4096, 512]   natural (stationary [K=H, M=dims])
    wkv  [4096, 256]   wk|wv column-concat
    wo   [512, 4096]   natural (moving operand)
    cosf [128, 1024]   freqs_cos.T stacked twice on the partition axis
    sinf [128, 1024]   [-freqs_sin.T ; +freqs_sin.T]
    out  [2048, 4096]  partial output (bf16; host sums in fp32)
"""

import numpy as np
import ml_dtypes

import concourse.bass as bass
import concourse.mybir as mybir
import concourse.tile as tile
from concourse import bacc
from concourse.bass_utils import run_bass_kernel_spmd
from concourse.masks import make_identity

F32 = mybir.dt.float32
BF16 = mybir.dt.bfloat16

B = 2
S = 1024          # tokens per batch
H = 4096          # model dim
D = 128           # head dim
HQ = 4            # q heads per core
NT = B * S        # total tokens
SCALE = 1.0 / float(np.sqrt(D))
NEG = -1.0e30     # additive causal mask value (pre-scale)

HC = H // 128     # 32 contraction chunks for the projections
WARMUP_MMS = 26   # dummy matmuls to warm the HAM clock gate


def build_program():
    nc = bacc.Bacc("TRN2", target_bir_lowering=False, debug=False, num_devices=8)

    xT = nc.dram_tensor("xT", [H, NT], BF16, kind="ExternalInput").ap()
    wq = nc.dram_tensor("wq", [H, HQ * D], BF16, kind="ExternalInput").ap()
    wkv = nc.dram_tensor("wkv", [H, 2 * D], BF16, kind="ExternalInput").ap()
    wo = nc.dram_tensor("wo", [HQ * D, H], BF16, kind="ExternalInput").ap()
    cosf = nc.dram_tensor("cosf", [128, S], BF16, kind="ExternalInput").ap()
    sinf = nc.dram_tensor("sinf", [128, S], BF16, kind="ExternalInput").ap()
    out = nc.dram_tensor("out", [NT, H], BF16, kind="ExternalOutput").ap()

    xT_r = xT.rearrange("(n p) f -> p n f", p=128)     # [128, 32, 2048]
    wq_r = wq.rearrange("(n p) f -> p n f", p=128)     # [128, 32, 512]
    wkv_r = wkv.rearrange("(n p) f -> p n f", p=128)   # [128, 32, 256]
    wo_r = wo.rearrange("(n p) f -> p n f", p=128)     # [128, 4, 4096]

    with tile.TileContext(nc) as tc:
        with (
            tc.tile_pool(name="const", bufs=1) as const,
            tc.tile_pool(name="weights", bufs=1) as weights,
            tc.tile_pool(name="stream", bufs=6) as stream,
            tc.tile_pool(name="acts", bufs=1) as acts,
            tc.tile_pool(name="vts", bufs=2) as vts,
            tc.tile_pool(name="ropes", bufs=4) as ropes,
            tc.tile_pool(name="probs", bufs=3) as probs,
            tc.tile_pool(name="an", bufs=6) as anpool,
            tc.tile_pool(name="ev", bufs=3) as evpool,
            tc.tile_pool(name="stats", bufs=16) as stats,
            tc.tile_pool(name="ps", bufs=1, space="PSUM") as psum,
        ):
            # ---- tiny constant first: warmup matmul source -----------------
            warm = const.tile([128, 128], BF16)
            nc.gpsimd.memset(warm, 0.25)

            # ---- PSUM bank map (all persistent, manually sliced) -----------
            psq = [psum.tile([128, 512], F32, tag=f"psq{i}", name=f"psq{i}")
                   for i in range(HQ)]
            psk = psum.tile([128, 512], F32, tag="psk")
            psv = psum.tile([128, 512], F32, tag="psv")
            stb = psum.tile([128, 512], F32, tag="st")
            pab = psum.tile([128, 512], F32, tag="pa")
            PA_SL = [slice(0, D + 1), slice(272, 272 + D + 1)]
            TP_SL = slice(136, 264)          # f32 transpose slot in pab

            # ---- warmup: keep PE busy from the end of the preamble so the
            # HAM clock-gate reaches 8/8 before the first real matmul --------
            for i in range(WARMUP_MMS):
                nc.tensor.matmul(stb[:, 0:128], warm, warm,
                                 start=True, stop=True)

            # ---- constants -------------------------------------------------
            ident = const.tile([128, 128], BF16)
            make_identity(nc, ident)
            identf = const.tile([128, 128], F32)
            nc.vector.tensor_copy(identf, ident)

            # maskT[p, f] = 0 where p <= f (k <= q valid), NEG where k > q
            maskT = const.tile([128, 128], F32)
            nc.gpsimd.memset(maskT, 0.0)
            nc.gpsimd.affine_select(
                out=maskT,
                in_=maskT,
                compare_op=mybir.AluOpType.is_ge,
                fill=NEG,
                base=0,
                pattern=[[1, 128]],
                channel_multiplier=-1,
            )

            # ---- resident weights (DMAs emitted just-in-time) --------------
            wq_s = weights.tile([128, HC, HQ * D], BF16)
            wkv_s = weights.tile([128, HC, 2 * D], BF16)
            cosf_s = const.tile([128, S], BF16)
            sinf_s = const.tile([128, S], BF16)
            wo_s = weights.tile([128, HQ, H], BF16)

            # ---- persistent activations ------------------------------------
            qTs, kTs, vnats, attnTs = [], [], [], []
            for b in range(B):
                qTs.append(acts.tile([128, HQ, S], BF16, tag=f"qT{b}",
                                     name=f"qT{b}"))
                kTs.append(acts.tile([128, S], BF16, tag=f"kT{b}",
                                     name=f"kT{b}"))
                vnats.append(acts.tile([128, S // 128, D + 1], BF16,
                                       tag=f"vnat{b}", name=f"vnat{b}"))
                attnTs.append(acts.tile([128, HQ, S], BF16, tag=f"attnT{b}",
                                        name=f"attnT{b}"))
            for b in range(B):
                nc.gpsimd.memset(vnats[b], 1.0)  # ones column for the denom

            # ============================================================
            # RoPE helpers
            # ============================================================
            def rope_swap(dst, nh, tsl):
                """Half-swap of dst[:, 0:nh, tsl] into a scratch tile via
                small DMAs (partition halves cross); split per 2 heads to
                keep per-transfer latency low."""
                scr = ropes.tile([128, HQ, 256], BF16, tag="scr")
                for g0 in range(0, nh, 2):
                    g1 = min(g0 + 2, nh)
                    nc.sync.dma_start(out=scr[0:64, g0:g1, :],
                                      in_=dst[64:128, g0:g1, tsl])
                    nc.sync.dma_start(out=scr[64:128, g0:g1, :],
                                      in_=dst[0:64, g0:g1, tsl])
                return scr

            def rope_muls(dst, d, tsl, scr):
                nc.gpsimd.tensor_mul(dst[:, d, tsl], dst[:, d, tsl],
                                     cosf_s[:, tsl])
                nc.gpsimd.tensor_mul(scr[:, d, :], scr[:, d, :],
                                     sinf_s[:, tsl])
                nc.gpsimd.tensor_add(dst[:, d, tsl], dst[:, d, tsl],
                                     scr[:, d, :])

            # ============================================================
            # attention step generators
            # ============================================================
            st_ctr = [0]
            pa_ctr = [0]

            def attn_unit_steps(b, hh, qc):
                """Yield per small PE step for attention unit (batch b,
                head hh, 512-token q-chunk qc).  Scores run in 256-wide
                q-windows through the stb ping-pong; PV accumulates into
                pab slots; transposes go through the pab transpose slot."""
                pt = probs.tile([128, S // 128, 512], BF16, tag="pt")
                q0 = qc * 512

                def score1(kc, w):
                    qw0 = q0 + w * 256
                    qoff = max(0, kc * 128 - qw0)  # 0 or 128 (diag)
                    sl = st_ctr[0] % 2
                    st_ctr[0] += 1
                    st = stb[:, sl * 256:sl * 256 + 256]
                    nc.tensor.matmul(
                        st[:, qoff:256],
                        kTs[b][:, kc * 128:(kc + 1) * 128],
                        qTs[b][:, hh, qw0 + qoff:qw0 + 256],
                        start=True, stop=True,
                    )
                    if kc * 128 >= qw0:  # diagonal block: causal mask
                        nc.vector.tensor_add(
                            st[:, qoff:qoff + 128],
                            st[:, qoff:qoff + 128],
                            maskT,
                        )
                    nc.scalar.activation(
                        pt[:, kc, w * 256 + qoff:w * 256 + 256],
                        st[:, qoff:256],
                        mybir.ActivationFunctionType.Exp,
                        scale=SCALE,
                    )

                def pv(qr):
                    qb = qc * 4 + qr
                    sl = pa_ctr[0] % 2
                    pa_ctr[0] += 1
                    pa = pab[:, PA_SL[sl]]
                    for kc in range(qb + 1):
                        nc.tensor.matmul(
                            pa,
                            pt[:, kc, qr * 128:(qr + 1) * 128],
                            vnats[b][:, kc, :],
                            start=(kc == 0), stop=(kc == qb),
                        )
                    rec = stats.tile([128, 1], F32, tag="stt")
                    nc.vector.reciprocal(rec, pa[:, D:D + 1])
                    an = anpool.tile([128, 128], F32, tag="an")
                    nc.vector.tensor_scalar_mul(an, pa[:, 0:D], rec)
                    return an

                def tc1(qr):
                    # transpose + evacuation as ONE step: the tp slot is
                    # shared across concurrently-zippered units, so the
                    # copy must be emitted before any other step can
                    # overwrite the slot
                    nc.tensor.transpose(pab[:, TP_SL], ans[qr], identf)
                    nc.vector.tensor_copy(
                        attnTs[b][:, hh, q0 + qr * 128:q0 + (qr + 1) * 128],
                        pab[:, TP_SL],
                    )

                ans = {}
                mk = ans.__setitem__
                if qc == 0:
                    plan = [
                        lambda: score1(0, 0), lambda: score1(1, 0),
                        lambda: mk(0, pv(0)),
                        lambda: score1(0, 1),
                        lambda: mk(1, pv(1)),
                        lambda: tc1(0),
                        lambda: score1(1, 1),
                        lambda: tc1(1),
                        lambda: score1(2, 1),
                        lambda: score1(3, 1),
                        lambda: mk(2, pv(2)),
                        lambda: tc1(2),
                        lambda: mk(3, pv(3)),
                        lambda: tc1(3),
                    ]
                else:
                    plan = (
                        [lambda kc=kc: score1(kc, 0) for kc in range(6)]
                        + [lambda: mk(0, pv(0)),
                           lambda: score1(0, 1),
                           lambda: mk(1, pv(1)),
                           lambda: tc1(0),
                           lambda: score1(1, 1),
                           lambda: score1(2, 1),
                           lambda: tc1(1),
                           lambda: score1(3, 1),
                           lambda: score1(4, 1),
                           lambda: score1(5, 1),
                           lambda: score1(6, 1),
                           lambda: mk(2, pv(2)),
                           lambda: tc1(2),
                           lambda: score1(7, 1),
                           lambda: mk(3, pv(3)),
                           lambda: tc1(3)]
                    )
                for stepfn in plan:
                    stepfn()
                    yield

            def zipper(ga, gb):
                """Alternate steps of two generators (keeps score-slot
                ping-pong distance >= 2 even in multi-step pulls)."""
                while True:
                    alive = False
                    for g in (ga, gb):
                        try:
                            next(g)
                            alive = True
                            yield
                        except StopIteration:
                            pass
                    if not alive:
                        return

            # ---- injector: pulls attention steps into host streams --------
            class Inject:
                def __init__(self):
                    self.q = []          # list of (gen, batch)
                    self.acc = 0.0
                    self.skip = 0

                def add(self, b, *its):
                    self.q.extend((g, b) for g in its)

                def delay(self, n):
                    self.skip += n

                def _step(self):
                    while self.q:
                        try:
                            next(self.q[0][0])
                            return True
                        except StopIteration:
                            self.q.pop(0)
                    return False

                def pull(self, rate):
                    if self.skip > 0:
                        self.skip -= 1
                        return
                    self.acc += rate
                    while self.acc >= 1.0:
                        self.acc -= 1.0
                        if not self._step():
                            return

                def flush_batch(self, b):
                    while self.q and self.q[0][1] == b:
                        if not self._step():
                            return

                def flush(self):
                    while self._step():
                        pass

            inj = Inject()

            # ============================================================
            # x stream: flat sequence of [128, 2, 512] tiles (2 hc each),
            # 16 per 512-token chunk, triggered 5 tiles ahead
            # ============================================================
            chunk_order = [(0, 0), (0, 1), (1, 0), (1, 1)]
            xp_tiles = {}
            xp_next = [0]     # next global tile index to trigger

            def xp_trigger(m):
                ci, j = divmod(m, 16)
                if ci >= len(chunk_order):
                    return
                b, t = chunk_order[ci]
                tok0 = b * S + t * 512
                xp = stream.tile([128, 2, 512], BF16, tag="xp",
                                 name=f"xp{m}")
                first = (ci == 0 and j < 3)
                if first:
                    # split the earliest tiles so their latency is small
                    # (one DMA engine sustains only ~21GB/s per transfer)
                    for h in range(2):
                        nc.sync.dma_start(
                            out=xp[:, h, 0:256],
                            in_=xT_r[:, 2 * j + h, tok0:tok0 + 256])
                        nc.scalar.dma_start(
                            out=xp[:, h, 256:512],
                            in_=xT_r[:, 2 * j + h, tok0 + 256:tok0 + 512])
                else:
                    for h in range(2):
                        nc.sync.dma_start(
                            out=xp[:, h, :],
                            in_=xT_r[:, 2 * j + h, tok0:tok0 + 512])
                xp_tiles[m] = xp

            def xp_ensure(m):
                while xp_next[0] <= m:
                    xp_trigger(xp_next[0])
                    xp_next[0] += 1

            # ============================================================
            # projection: qT/kT/vT = w.T @ x  (512-token chunks)
            # ============================================================
            evac_prev = [None]

            def run_evac():
                if evac_prev[0] is not None:
                    fn = evac_prev[0]
                    evac_prev[0] = None
                    fn()

            def proj_evac(b, t, rate):
                """Evacuate + rope chunk (b, t).  qT copies first (they
                gate the next chunk's matmuls); rope head-major."""
                tsl = slice(t * 512, (t + 1) * 512)
                h0 = slice(t * 512, t * 512 + 256)
                h1 = slice(t * 512 + 256, (t + 1) * 512)
                kT, qT = kTs[b], qTs[b]
                kTv = kT.rearrange("p (a f) -> p a f", a=1)
                nc.scalar.copy(qT[:, 0, tsl], psq[0])
                nc.scalar.copy(qT[:, 1, tsl], psq[1])
                nc.vector.tensor_copy(qT[:, 2, tsl], psq[2])
                nc.vector.tensor_copy(qT[:, 3, tsl], psq[3])
                nc.vector.tensor_copy(kT[:, tsl], psk)
                vT = vts.tile([128, 512], F32, tag="vT")
                nc.vector.tensor_copy(vT, psv)
                # v natural via the shared transpose slot
                for i in range(4):
                    nc.tensor.transpose(pab[:, TP_SL],
                                        vT[:, i * 128:(i + 1) * 128], identf)
                    nc.vector.tensor_copy(
                        vnats[b][:, t * 4 + i, 0:128], pab[:, TP_SL])
                sck0 = rope_swap(kTv, 1, h0)
                sck1 = rope_swap(kTv, 1, h1)
                scq0 = rope_swap(qT, HQ, h0)
                scq1 = rope_swap(qT, HQ, h1)
                rope_muls(kTv, 0, h0, sck0)
                rope_muls(kTv, 0, h1, sck1)
                for d in range(HQ):
                    rope_muls(qT, d, h0, scq0)
                    rope_muls(qT, d, h1, scq1)
                # pulls ONLY after the ropes are emitted: hosted attention
                # steps reading this chunk must order after the rotation
                for _ in range(6):
                    inj.pull(rate)

            def proj_chunk(b, t, rate):
                ci = chunk_order.index((b, t))
                first = ci == 0
                if first:
                    # first tiles + first weights split fine, interleaved,
                    # alternating queues: many small concurrent transfers
                    xp_ensure(1)
                    for hc in range(4):    # wq hc 0..3 in 128KB pieces
                        eng = nc.sync if hc % 2 == 0 else nc.scalar
                        eng.dma_start(out=wq_s[:, hc:hc + 1, :],
                                      in_=wq_r[:, hc:hc + 1, :])
                    nc.sync.dma_start(out=wkv_s[:, 0:4, :],
                                      in_=wkv_r[:, 0:4, :])
                    xp_ensure(3)
                    for p in range(2, 16):
                        eng = nc.sync if p % 2 == 0 else nc.scalar
                        eng.dma_start(out=wq_s[:, 2 * p:2 * p + 2, :],
                                      in_=wq_r[:, 2 * p:2 * p + 2, :])
                        if p == 5:
                            xp_ensure(4)
                        if p == 9:
                            xp_ensure(5)
                    nc.scalar.dma_start(out=wkv_s[:, 4:8, :],
                                        in_=wkv_r[:, 4:8, :])
                    for p in range(1, 4):
                        eng = nc.sync if p % 2 == 1 else nc.scalar
                        eng.dma_start(out=wkv_s[:, 8 * p:8 * p + 8, :],
                                      in_=wkv_r[:, 8 * p:8 * p + 8, :])
                    nc.scalar.dma_start(out=cosf_s, in_=cosf)
                    nc.scalar.dma_start(out=sinf_s, in_=sinf)
                xp_ensure(ci * 16 + 5)
                run_evac()
                for j in range(16):          # 16 x-tiles of 2 hc
                    xp_ensure(ci * 16 + j + 5)
                    xp = xp_tiles.pop(ci * 16 + j)
                    for sub in range(2):
                        hc = 2 * j + sub
                        fst, lst = hc == 0, hc == HC - 1
                        for dd in range(HQ):
                            nc.tensor.matmul(
                                psq[dd],
                                wq_s[:, hc, dd * 128:(dd + 1) * 128],
                                xp[:, sub, :], start=fst, stop=lst)
                        nc.tensor.matmul(psk, wkv_s[:, hc, 0:128],
                                         xp[:, sub, :], start=fst, stop=lst)
                        nc.tensor.matmul(psv, wkv_s[:, hc, 128:256],
                                         xp[:, sub, :], start=fst, stop=lst)
                    inj.pull(rate)
                evac_prev[0] = lambda: proj_evac(b, t, rate)

            # ============================================================
            # output projection: out[tok, :] += attnT.T @ wo
            # (po ring borrows the free proj q-banks)
            # ============================================================
            def out_batch(b, rate):
                tok0 = b * S
                attnT = attnTs[b]
                for tb in range(S // 128):
                    ev = evpool.tile([128, H], BF16, tag="ev")
                    for ncol in range(8):
                        po = psq[(tb * 8 + ncol) % 3]
                        for d in range(HQ):
                            nc.tensor.matmul(
                                po,
                                attnT[:, d, tb * 128:(tb + 1) * 128],
                                wo_s[:, d, ncol * 512:(ncol + 1) * 512],
                                start=(d == 0), stop=(d == HQ - 1),
                            )
                        if ncol % 2 == 0:
                            nc.scalar.copy(ev[:, ncol * 512:(ncol + 1) * 512],
                                           po)
                        else:
                            nc.vector.tensor_copy(
                                ev[:, ncol * 512:(ncol + 1) * 512], po)
                        inj.pull(rate)
                    nsp = 8 if (b == B - 1 and tb == S // 128 - 1) else 4
                    w = H // nsp
                    for i in range(nsp):
                        nc.sync.dma_start(
                            out=out[tok0 + tb * 128: tok0 + (tb + 1) * 128,
                                    i * w:(i + 1) * w],
                            in_=ev[:, i * w:(i + 1) * w],
                        )

            # ============================================================
            # schedule
            # ============================================================
            proj_chunk(0, 0, 0.0)
            # proj (0,1) hosts attn(b0) qc0: 4 units x 18 steps
            inj.add(0, zipper(attn_unit_steps(0, 0, 0),
                              attn_unit_steps(0, 1, 0)),
                    zipper(attn_unit_steps(0, 2, 0),
                           attn_unit_steps(0, 3, 0)))
            inj.delay(6)
            proj_chunk(0, 1, 3.0)
            # proj (1,0) + (1,1) host attn(b0) qc1 then attn(b1) qc0
            inj.add(0, zipper(attn_unit_steps(0, 0, 1),
                              attn_unit_steps(0, 1, 1)),
                    zipper(attn_unit_steps(0, 2, 1),
                           attn_unit_steps(0, 3, 1)))
            proj_chunk(1, 0, 3.0)
            for i in range(0, 4):
                nc.scalar.dma_start(
                    out=wo_s[:, :, i * 512:(i + 1) * 512],
                    in_=wo_r[:, :, i * 512:(i + 1) * 512])
            inj.add(1, zipper(attn_unit_steps(1, 0, 0),
                              attn_unit_steps(1, 1, 0)),
                    zipper(attn_unit_steps(1, 2, 0),
                           attn_unit_steps(1, 3, 0)))
            proj_chunk(1, 1, 3.0)
            for i in range(4, 8):
                nc.scalar.dma_start(
                    out=wo_s[:, :, i * 512:(i + 1) * 512],
                    in_=wo_r[:, :, i * 512:(i + 1) * 512])
            run_evac()          # final proj chunk evacuation
            # attn b0 must be fully emitted before out b0 reads attnT[0];
            # pending b1 steps continue into the out-phase hosting
            inj.flush_batch(0)
            inj.add(1, zipper(attn_unit_steps(1, 0, 1),
                              attn_unit_steps(1, 1, 1)),
                    zipper(attn_unit_steps(1, 2, 1),
                           attn_unit_steps(1, 3, 1)))
            out_batch(0, 1.8)
            inj.flush()
            out_batch(1, 0.0)

    nc.compile()
    return nc


_NC = None


def _get_nc():
    global _NC
    if _NC is None:
        _NC = build_program()
    return _NC


def make_in_maps(x, wq, wk, wv, wo, freqs_cos, freqs_sin):
    bf = ml_dtypes.bfloat16
    x = np.asarray(x, np.float32)
    xT = np.ascontiguousarray(x.reshape(NT, H).T.astype(bf))
    cosT = np.asarray(freqs_cos, np.float32).T
    sinT = np.asarray(freqs_sin, np.float32).T
    cosf = np.ascontiguousarray(np.concatenate([cosT, cosT], 0).astype(bf))
    sinf = np.ascontiguousarray(np.concatenate([-sinT, sinT], 0).astype(bf))
    wq = np.asarray(wq, np.float32).astype(bf)
    wk = np.asarray(wk, np.float32).astype(bf)
    wv = np.asarray(wv, np.float32).astype(bf)
    wo = np.asarray(wo, np.float32).astype(bf)
    in_maps = []
    for c in range(8):
        in_maps.append({
            "xT": xT,
            "wq": np.ascontiguousarray(wq[:, c * 512:(c + 1) * 512]),
            "wkv": np.ascontiguousarray(
                np.concatenate([wk[:, c * 128:(c + 1) * 128],
                                wv[:, c * 128:(c + 1) * 128]], axis=1)),
            "wo": np.ascontiguousarray(wo[c * 512:(c + 1) * 512, :]),
            "cosf": cosf,
            "sinf": sinf,
        })
    return in_maps


def kernel(x, wq, wk, wv, wo, freqs_cos, freqs_sin, start_pos=0, **_):
    nc = _get_nc()
    in_maps = make_in_maps(x, wq, wk, wv, wo, freqs_cos, freqs_sin)
    res = run_bass_kernel_spmd(nc, in_maps, list(range(8)))
    acc = res.results[0]["out"].astype(np.float32)
    for c in range(1, 8):
        acc = acc + res.results[c]["out"].astype(np.float32)
    return acc.reshape(B, S, H)
